# revision 76
# baseline (speedup 1.0000x reference)
"""DNeRF renderer on 8 Trainium2 cores (Bass/Tile) — v2.

Data-parallel over rays (1024 rays/core, 8 ray-tiles of 128 rays).

v2 structure (vs v1):
- All big matmuls in float32r (1 cyc/row on the PE, full-rate fp32).
- Two-phase schedule: phase 1 = coarse MLP + sigma for all 8 ray-tiles,
  then the per-ray sort/searchsorted machinery BATCHED over pairs of
  ray-tiles (segmented scans via (mult,add)-reset tricks), then phase 2 =
  fine MLP + compositing per ray-tile.  PE streams matmuls back-to-back
  while DVE/Act/Pool run the machinery of earlier tiles.
- Sigma eviction packed into [128,512] psum via tile_position 32-blocks
  (free-size-bound engine cost drops 4x).
- Layer-1 bias folded into the relu eviction (per-partition bias column),
  dropping the ones-row from the matmul rhs.
- bc2 bias folded into the rgb eviction copies.
- dterm broadcast-add runs on the (otherwise idle) Pool engine.
"""

import numpy as np
import ml_dtypes
from contextlib import ExitStack

import concourse.bass as bass
import concourse.bacc as bacc
import concourse.mybir as mybir
import concourse.tile as tile
from concourse.bass_utils import run_bass_kernel_spmd
from concourse import library_config

dt = mybir.dt
Alu = mybir.AluOpType
Act = mybir.ActivationFunctionType
AxX = mybir.AxisListType.X

NCORES = 8
NRAYS = 8192
R = NRAYS // NCORES      # rays per core
P = 128                  # rays per tile (partitions)
T = R // P               # ray-tiles per core
S = 64                   # num_steps
U = 64                   # upsample_steps
M = S + U                # merged samples
MB = 2                   # ray-tiles per machinery batch
NG = T // MB
MIN_NEAR = 0.05
M24 = 16777216.0         # 2^24

_BUILT = None
SIM_SAFE = False


def _build():
    nc = bacc.Bacc("TRN2", target_bir_lowering=False, debug=False,
                   num_devices=NCORES)

    def din(name, shape, dtype=dt.float32):
        return nc.dram_tensor(name, shape, dtype, kind="ExternalInput").ap()

    rdT6_in = din("rdT6_k", [T, 6, P // 2 * M], dt.bfloat16)
    c_all_in = din("c_all_k", [P, 4959])
    wbig_in = din("wbig", [128, 136], dt.bfloat16)
    lb6_in = din("lb6", [6, 256], dt.bfloat16)
    bcols_in = din("bcols", [128, 2])
    lhsT6 = din("lhsT6", [6, 128], dt.float32r)
    bc2col = din("bc2col", [6, 1])

    img_out = nc.dram_tensor("img_k", [P, T, 3], dt.float32,
                             kind="ExternalOutput").ap()


    def dep0(ap_):
        # partition-strided APs confuse subtile dep tracking; anchor the
        # tracked range at offset 0 so read/write overlap is detected.
        return bass.AP(tensor=ap_.tensor, offset=ap_.offset, ap=ap_.ap,
                       dep_tracking_offset=0)

    def mmr(out, lhsT, rhs, **kw):
        nc.tensor.matmul(out, lhsT.bitcast(dt.float32r),
                         rhs.bitcast(dt.float32r), **kw)

    with tile.TileContext(nc) as tc, ExitStack() as ctx:

        cpool = ctx.enter_context(tc.tile_pool(name="consts", bufs=1))
        spool = ctx.enter_context(tc.tile_pool(name="setup", bufs=1))
        mpool = ctx.enter_context(tc.tile_pool(name="mach", bufs=1))
        wpool = ctx.enter_context(tc.tile_pool(name="work", bufs=2))
        rpool = ctx.enter_context(tc.tile_pool(name="rhs", bufs=1))
        rpoolf = ctx.enter_context(tc.tile_pool(name="rhsf", bufs=2))
        bpool = ctx.enter_context(tc.tile_pool(name="big", bufs=2))
        gpool = ctx.enter_context(tc.tile_pool(name="gather", bufs=1))
        kpool = ctx.enter_context(tc.tile_pool(name="comp", bufs=1))
        ppA = ctx.enter_context(tc.tile_pool(name="psA", bufs=2, space="PSUM"))
        ppG = ctx.enter_context(tc.tile_pool(name="psG", bufs=2, space="PSUM"))
        ppS = ctx.enter_context(tc.tile_pool(name="psS", bufs=2, space="PSUM"))
        ppC = ctx.enter_context(tc.tile_pool(name="psC", bufs=2, space="PSUM"))
        dpool = ctx.enter_context(tc.tile_pool(name="dram", bufs=3, space="DRAM"))

        def cload(ap_in, shape, tag, dtype=dt.float32):
            t_ = cpool.tile(shape, dtype, tag=tag, name=tag)
            nc.sync.dma_start(t_[:], ap_in)
            return t_

        c_all = cload(c_all_in, [P, 4959], tag='c_all')
        wbig = cload(wbig_in, [128, 136], tag='c_wbig', dtype=dt.bfloat16)
        lb6 = cload(lb6_in, [6, 256], tag='c_lb6', dtype=dt.bfloat16)
        bcols = cload(bcols_in, [128, 2], tag='c_bcols')
        lhsT6_s = cload(lhsT6, [6, 128], tag='c_lhsT6', dtype=dt.float32r)
        bc2_s = cload(bc2col, [6, 1], tag='c_bc2col')
        v128_s = c_all[:, 0:64]
        cc_s = c_all[:, 64:72]
        smM_s = c_all[:, 72:1096]
        iop1_s = c_all[:, 1096:2120]
        s256_s = c_all[:, 2120:3144]
        sm62_s = c_all[:, 3144:3640]
        iev_s = c_all[:, 3640:4136]
        oneS_s = c_all[:, 4136:4648]
        one2M_s = c_all[:, 4648:4904]
        bg_s = c_all[:, 4904:4907]
        scl_s = c_all[:, 4907:4911]
        ro_s = c_all[:, 4911:4935].rearrange("p (t c) -> p t c", t=T)
        rd_s = c_all[:, 4935:4959].rearrange("p (t c) -> p t c", t=T)
        w0p_s = wbig[:, 0:2]
        wgc_s = wbig[:, 2:130]
        wc2_s = wbig[:, 130:136]
        lhsT6b_s = lb6[:, 0:128]
        dlhs6_s = lb6[:, 128:256]
        b1_s = bcols[:, 0:1]
        bch1_s = bcols[:, 1:2]

        ones_c = cc_s[:, 0:1]
        eps_c = cc_s[:, 1:2]
        e15_c = cc_s[:, 2:3]
        e5_c = cc_s[:, 3:4]
        nhalf_c = cc_s[:, 4:5]
        m24_c = cc_s[:, 5:6]
        nm24_c = cc_s[:, 6:7]
        n8003_c = cc_s[:, 7:8]
        bd2_0c = scl_s[:, 0:1]

        def bc(col, n):
            return col.broadcast_to((P, n))

        # ============ STAGE A: ray setup (batched over T) ============
        n24 = T * 3

        def st(shape, tag, dtype=dt.float32):
            return spool.tile(shape, dtype, tag=tag, name=tag)

        negd = st([P, T, 3], 's_negd')
        nc.vector.tensor_scalar(negd[:], rd_s[:], -1.0, None, Alu.mult)
        absd = st([P, T, 3], 's_absd')
        nc.vector.tensor_tensor(absd[:], rd_s[:], negd[:], Alu.max)
        dmask = st([P, T, 3], 's_dmask', dt.uint8)
        nc.vector.tensor_scalar(dmask[:], absd[:], 1e-9, None, Alu.is_lt)
        dsafe = st([P, T, 3], 's_dsafe')
        nc.vector.select(dsafe[:].rearrange("p t c -> p (t c)"),
                         dmask[:].rearrange("p t c -> p (t c)"),
                         bc(eps_c, n24),
                         rd_s[:].rearrange("p t c -> p (t c)"))
        invd = st([P, T, 3], 's_invd')
        nc.vector.reciprocal(invd[:], dsafe[:])
        a1 = st([P, T, 3], 's_a1')
        nc.vector.scalar_tensor_tensor(a1[:], ro_s[:], 1.0, invd[:],
                                       Alu.add, Alu.mult)
        b1 = st([P, T, 3], 's_b1')
        nc.vector.scalar_tensor_tensor(b1[:], ro_s[:], -1.0, invd[:],
                                       Alu.add, Alu.mult)
        mx = st([P, T, 3], 's_mx')
        nc.vector.tensor_tensor(mx[:], a1[:], b1[:], Alu.max)
        mn = st([P, T, 3], 's_mn')
        nc.vector.tensor_tensor(mn[:], a1[:], b1[:], Alu.min)
        tmin = st([P, T], 's_tmin')
        nc.vector.tensor_reduce(tmin[:], mx[:], AxX, Alu.min)
        tmax = st([P, T], 's_tmax')
        nc.vector.tensor_reduce(tmax[:], mn[:], AxX, Alu.max)
        near = st([P, T], 's_near')
        nc.vector.tensor_scalar(near[:], tmin[:], -1.0, MIN_NEAR,
                                Alu.mult, Alu.max)
        tmaxt = st([P, T], 's_tmaxt')
        nc.vector.tensor_scalar(tmaxt[:], tmax[:], -1.0, None, Alu.mult)
        fmask = st([P, T], 's_fmask', dt.uint8)
        nc.vector.tensor_tensor(fmask[:], tmaxt[:], near[:], Alu.is_lt)
        nearp = st([P, T], 's_nearp')
        nc.vector.tensor_scalar(nearp[:], near[:], 1e-2, None, Alu.add)
        far = st([P, T], 's_far')
        nc.vector.select(far[:], fmask[:], nearp[:], tmaxt[:])
        rng = st([P, T], 's_rng')
        nc.vector.tensor_tensor(rng[:], far[:], near[:], Alu.subtract)
        dzv = st([P, T], 's_dzv')
        nc.vector.tensor_scalar(dzv[:], rng[:], 1.0 / 63.0, None, Alu.mult)
        sdv = st([P, T], 's_sdv')
        nc.vector.tensor_scalar(sdv[:], rng[:], 1.0 / 64.0, None, Alu.mult)
        invdz = st([P, T], 's_invdz')
        nc.vector.reciprocal(invdz[:], dzv[:])
        inv2dz = st([P, T], 's_inv2dz')
        nc.vector.tensor_scalar(inv2dz[:], invdz[:], 2.0, None, Alu.mult)
        mid0 = st([P, T], 's_mid0')
        nc.vector.scalar_tensor_tensor(mid0[:], dzv[:], 0.5, near[:],
                                       Alu.mult, Alu.add)
        aoff = st([P, T], 's_aoff')
        nc.vector.tensor_tensor(aoff[:], near[:], dzv[:], Alu.subtract)

        # coarse z grid, batched: zc3[p,t,s] = near[p,t] + v128[s]*rng[p,t]
        zc3 = st([P, T, S], 's_zc3')
        v3 = v128_s[:].rearrange("p (o s) -> p o s", o=1).broadcast_to((P, T, S))
        rng3 = rng[:].rearrange("p (t o) -> p t o", o=1).broadcast_to((P, T, S))
        near3 = near[:].rearrange("p (t o) -> p t o", o=1).broadcast_to((P, T, S))
        nc.vector.tensor_tensor(zc3[:], v3, rng3, Alu.mult)
        nc.vector.tensor_tensor(zc3[:], zc3[:], near3, Alu.add)

        # persistent cross-phase arrays
        h20T = st([P, T, S], 's_h20T')
        Z3 = st([P, T, M], 's_Z3')
        dl3 = st([P, T, M], 's_dl3')
        img_all = st([P, T, 3], 's_img')

        def b3(col2, n):
            # [P, MB] -> [P, MB, n] broadcast
            return col2.rearrange("p (t o) -> p t o", o=1) \
                .broadcast_to((P, MB, n))

        # ================= machinery (batched per MB tiles) =================
        def mt(shape, tag, dtype=dt.float32):
            return mpool.tile(shape, dtype, tag=tag, name=tag)

        def do_mach(mb):
            t0 = mb * MB
            colM = slice(t0 * M, (t0 + MB) * M)
            colS = slice(t0 * S, (t0 + MB) * S)
            col62 = slice(t0 * 62, (t0 + MB) * 62)
            h20v = h20T[:, t0:t0 + MB, :]
            dz_sl = dzv[:, t0:t0 + MB]
            sd_sl = sdv[:, t0:t0 + MB]
            near_sl = near[:, t0:t0 + MB]
            i2dz_sl = inv2dz[:, t0:t0 + MB]
            mid0_sl = mid0[:, t0:t0 + MB]
            aoff_sl = aoff[:, t0:t0 + MB]

            TM2 = MB * M

            # --- coarse composite weights ---
            sig3 = mt([P, MB, S], 'm_sig3')
            nc.scalar.activation(sig3[:].rearrange("p t s -> p (t s)"),
                                 h20v.rearrange("p t s -> p (t s)"),
                                 Act.Exp, bias=bd2_0c)
            dsgc = mt([P, MB, S], 'm_dsgc')
            nc.vector.tensor_tensor(dsgc[:], sig3[:], b3(dz_sl, S), Alu.mult)
            nc.vector.tensor_tensor(dsgc[:, :, S - 1:S], sig3[:, :, S - 1:S],
                                    b3(sd_sl, 1), Alu.mult)
            emc = mt([P, MB, S], 'm_emc')
            nc.scalar.activation(emc[:].rearrange("p t s -> p (t s)"),
                                 dsgc[:].rearrange("p t s -> p (t s)"),
                                 Act.Exp, scale=-1.0)
            d0c = mt([P, MB, S], 'm_d0c')
            nc.vector.memset(d0c[:, :, 0:1], 0.0)
            nc.scalar.activation(d0c[:, :, 1:S], emc[:, :, 0:S - 1],
                                 Act.Identity, bias=e15_c)
            Tc = mt([P, MB, S], 'm_Tc')
            nc.vector.tensor_tensor_scan(
                Tc[:].rearrange("p t s -> p (t s)"),
                d0c[:].rearrange("p t s -> p (t s)"),
                oneS_s[:, colS], 0.0, Alu.mult, Alu.add)
            alpha = mt([P, MB, S], 'm_alpha')
            nc.scalar.activation(alpha[:].rearrange("p t s -> p (t s)"),
                                 emc[:].rearrange("p t s -> p (t s)"),
                                 Act.Identity, scale=-1.0, bias=ones_c)
            wts = mt([P, MB, S], 'm_wts')
            nc.vector.tensor_tensor(wts[:], alpha[:], Tc[:], Alu.mult)

            # --- pdf/cdf over weights[:,1:63] ---
            wp = mt([P, MB, 62], 'm_wp')
            nc.scalar.activation(wp[:], wts[:, :, 1:63], Act.Identity,
                                 bias=e5_c)
            ssum = mt([P, MB], 'm_ssum')
            nc.vector.tensor_reduce(ssum[:], wp[:], AxX, Alu.add)
            pinv = mt([P, MB], 'm_pinv')
            nc.vector.reciprocal(pinv[:], ssum[:])
            pdf = mt([P, MB, 62], 'm_pdf')
            nc.vector.tensor_tensor(pdf[:], wp[:], b3(pinv[:], 62), Alu.mult)
            cdf = mt([P, MB, 62], 'm_cdf')
            nc.vector.tensor_tensor_scan(
                cdf[:].rearrange("p t s -> p (t s)"), sm62_s[:, col62],
                pdf[:].rearrange("p t s -> p (t s)"), 0.0, Alu.mult, Alu.add)

            # --- scatter cdf onto per-segment 128-slot timelines ---
            r2 = mt([P, MB, 62], 'm_r2')
            r2f = r2[:].rearrange("p t s -> p (t s)")
            cdff = cdf[:].rearrange("p t s -> p (t s)")
            nc.scalar.activation(r2f, cdff, Act.Identity, scale=128.0,
                                 bias=m24_c)
            nc.scalar.activation(r2f, r2f, Act.Identity, bias=nm24_c)
            idx2f = mt([P, MB, 124], 'm_idx2f')
            i4 = idx2f[:].rearrange("p t (a b) -> p t a b", b=2)
            ev = i4[:, :, :, 0:1].rearrange("p t a b -> p t (a b)")
            od = i4[:, :, :, 1:2].rearrange("p t a b -> p t (a b)")
            nc.vector.tensor_tensor(
                ev, r2[:], iev_s[:, col62].rearrange("p (t s) -> p t s", t=MB),
                Alu.add)
            nc.scalar.activation(od, ev, Act.Identity, bias=ones_c)
            idx2i = mt([P, MB * 124], 'm_idx2i', dt.int16)
            nc.scalar.copy(idx2i[:], idx2f[:].rearrange("p t s -> p (t s)"))
            tlc2 = mt([P, MB * 256], 'm_tlc2', dt.int16)
            nc.gpsimd.local_scatter(tlc2[:], cdff.bitcast(dt.int16),
                                    idx2i[:], channels=P,
                                    num_elems=MB * 256, num_idxs=MB * 124)
            tlc = tlc2[:].bitcast(dt.float32)
            tlc3 = tlc.rearrange("p (t s) -> p t s", t=MB)

            # --- fills and counts on the timeline ---
            notC = mt([P, MB, M], 'm_notC')
            notCf = notC[:].rearrange("p t s -> p (t s)")
            nc.vector.tensor_scalar(notCf, tlc, 0.0, None, Alu.is_equal)
            notCp = mt([P, MB * M], 'm_notCp')
            nc.gpsimd.tensor_tensor(notCp[:], notCf, smM_s[:, colM], Alu.mult)
            kp1 = mt([P, MB, M], 'm_kp1')
            kp1f = kp1[:].rearrange("p t s -> p (t s)")
            nc.vector.tensor_tensor_scan(kp1f, smM_s[:, colM], notCf,
                                         0.0, Alu.mult, Alu.add)
            uu = mt([P, MB * M], 'm_uu')
            nc.scalar.activation(uu[:], kp1f, Act.Identity,
                                 scale=1.0 / 64.0, bias=nhalf_c)
            cntC = mt([P, MB * M], 'm_cntC')
            nc.vector.tensor_tensor(cntC[:], iop1_s[:, colM], kp1f,
                                    Alu.subtract)
            ffwd = mt([P, MB * M], 'm_ffwd')
            nc.vector.tensor_tensor_scan(ffwd[:], notCp[:], tlc, 0.0,
                                         Alu.mult, Alu.add)
            rnotC = mt([P, MB, M], 'm_rnotC')
            nc.scalar.copy(rnotC[:], notC[:, :, ::-1])
            rnotCp = mt([P, MB * M], 'm_rnotCp')
            nc.gpsimd.tensor_tensor(rnotCp[:],
                                    rnotC[:].rearrange("p t s -> p (t s)"),
                                    smM_s[:, colM], Alu.mult)
            rtlc = mt([P, MB, M], 'm_rtlc')
            nc.scalar.copy(rtlc[:], tlc3[:, :, ::-1])
            rbwd = mt([P, MB, M], 'm_rbwd')
            nc.vector.tensor_tensor_scan(
                rbwd[:].rearrange("p t s -> p (t s)"), rnotCp[:],
                rtlc[:].rearrange("p t s -> p (t s)"), 0.0, Alu.mult, Alu.add)

            # --- inverse-CDF lerp at u slots ---
            den = mt([P, MB, M], 'm_den')
            nc.vector.tensor_tensor(den[:], rbwd[:, :, ::-1],
                                    ffwd[:].rearrange("p (t s) -> p t s", t=MB),
                                    Alu.subtract)
            denf = den[:].rearrange("p t s -> p (t s)")
            mkd = mt([P, MB * M], 'm_mkd', dt.uint8)
            nc.vector.tensor_scalar(mkd[:], denf, 1e-5, None, Alu.is_lt)
            nc.vector.select(denf, mkd[:], bc(ones_c, TM2), denf)
            rden = mt([P, MB * M], 'm_rden')
            nc.vector.reciprocal(rden[:], denf)
            ttv = mt([P, MB * M], 'm_ttv')
            nc.vector.tensor_tensor(ttv[:], uu[:], ffwd[:], Alu.subtract)
            nc.vector.tensor_tensor(ttv[:], ttv[:], rden[:], Alu.mult)
            bg0 = mt([P, MB, M], 'm_bg0')
            nc.gpsimd.tensor_tensor(bg0[:],
                                    cntC[:].rearrange("p (t s) -> p t s", t=MB),
                                    b3(dz_sl, M), Alu.mult)
            nc.gpsimd.tensor_tensor(bg0[:], bg0[:], b3(mid0_sl, M), Alu.add)
            nz = mt([P, MB, M], 'm_nz')
            nc.vector.tensor_tensor(nz[:],
                                    ttv[:].rearrange("p (t s) -> p t s", t=MB),
                                    b3(dz_sl, M), Alu.mult)
            nc.vector.tensor_tensor(nz[:], nz[:], bg0[:], Alu.add)
            nzf = nz[:].rearrange("p t s -> p (t s)")

            # --- merge ranks into final (coarse ∪ fine) timeline ---
            q2 = mt([P, MB, M], 'm_q2')
            nc.vector.tensor_tensor(q2[:], nz[:], b3(near_sl, M), Alu.subtract)
            nc.vector.tensor_tensor(q2[:], q2[:], b3(i2dz_sl, M), Alu.mult)
            q2f = q2[:].rearrange("p t s -> p (t s)")
            nc.vector.tensor_scalar(q2f, q2f, 1.0, M24, Alu.add, Alu.add)
            nc.vector.tensor_scalar(q2f, q2f, M24, 0.0, Alu.subtract, Alu.max)
            nc.vector.tensor_scalar(q2f, q2f, 126.0, 8001.0, Alu.min, Alu.add)
            tk2 = mt([P, MB * M], 'm_tk2')
            nc.scalar.activation(tk2[:], kp1f, Act.Identity, scale=2.0,
                                 bias=n8003_c)
            mk2 = mt([P, MB * M], 'm_mk2')
            nc.vector.tensor_scalar(mk2[:], kp1f, 64.5, None, Alu.is_gt)
            minv = mt([P, MB * M], 'm_minv')
            nc.vector.scalar_tensor_tensor(minv[:], mk2[:], 1.0, notCf,
                                           Alu.add, Alu.subtract)
            m2 = mt([P, MB * M], 'm_m2')
            nc.vector.scalar_tensor_tensor(m2[:], minv[:], -4000.0, q2f,
                                           Alu.mult, Alu.add)
            ms = mt([P, MB * M], 'm_ms')
            nc.vector.tensor_tensor_scan(ms[:], smM_s[:, colM], m2[:],
                                         0.0, Alu.mult, Alu.max)
            rkv = mt([P, MB * M], 'm_rkv')
            nc.vector.tensor_tensor(rkv[:], tk2[:], ms[:], Alu.add)
            nc.vector.tensor_scalar(rkv[:], rkv[:], 254.0, None, Alu.min)
            nc.vector.scalar_tensor_tensor(rkv[:], minv[:], -4000.0, rkv[:],
                                           Alu.mult, Alu.add)
            fidx2f = mt([P, MB, 256], 'm_fidx2f')
            f4 = fidx2f[:].rearrange("p t (a b) -> p t a b", b=2)
            fev = f4[:, :, :, 0:1].rearrange("p t a b -> p t (a b)")
            fod = f4[:, :, :, 1:2].rearrange("p t a b -> p t (a b)")
            nc.vector.tensor_tensor(
                fev, rkv[:].rearrange("p (t s) -> p t s", t=MB),
                s256_s[:, colM].rearrange("p (t s) -> p t s", t=MB), Alu.add)
            nc.scalar.activation(fod, fev, Act.Identity, bias=ones_c)
            fidx2i = mt([P, MB * 256], 'm_fidx2i', dt.int16)
            nc.scalar.copy(fidx2i[:], fidx2f[:].rearrange("p t s -> p (t s)"))
            zf2 = mt([P, MB * 256], 'm_zf2', dt.int16)
            nc.gpsimd.local_scatter(zf2[:], nzf.bitcast(dt.int16),
                                    fidx2i[:], channels=P,
                                    num_elems=MB * 256, num_idxs=MB * 256)
            zsc = zf2[:].bitcast(dt.float32)

            # --- fill coarse slots with uniform grid ---
            isCC = mt([P, MB, M], 'm_isCC')
            isCCf = isCC[:].rearrange("p t s -> p (t s)")
            nc.vector.tensor_scalar(isCCf, zsc, 0.0, None, Alu.is_equal)
            cum2 = mt([P, MB, M], 'm_cum2')
            nc.vector.tensor_tensor_scan(
                cum2[:].rearrange("p t s -> p (t s)"), smM_s[:, colM],
                isCCf, 0.0, Alu.mult, Alu.add)
            zcf = mt([P, MB, M], 'm_zcf')
            nc.gpsimd.tensor_tensor(zcf[:], cum2[:], b3(dz_sl, M), Alu.mult)
            nc.gpsimd.tensor_tensor(zcf[:], zcf[:], b3(aoff_sl, M), Alu.add)
            Zv = Z3[:, t0:t0 + MB, :]
            nc.vector.tensor_tensor(Zv, isCC[:], zcf[:], Alu.mult)
            nc.vector.tensor_tensor(
                Zv, Zv, zsc.rearrange("p (t s) -> p t s", t=MB), Alu.add)
            dv = dl3[:, t0:t0 + MB, :]
            nc.vector.tensor_tensor(dv[:, :, 0:M - 1], Zv[:, :, 1:M],
                                    Zv[:, :, 0:M - 1], Alu.subtract)
            nc.scalar.copy(dv[:, :, M - 1:M],
                           sd_sl.rearrange("p (t o) -> p t o", o=1))


        # ================= PHASE 1: coarse MLP + sigma =================
        def phase1_tile(t):
            xyzc = wpool.tile([P, 3, S], dt.float32r, tag="xyzc", name="xyzc")
            for c in range(3):
                nc.gpsimd.tensor_tensor(xyzc[:, c, :], zc3[:, t, :],
                                        bc(rd_s[:, t, c:c + 1], S), Alu.mult)
                nc.gpsimd.tensor_tensor(xyzc[:, c, :], xyzc[:, c, :],
                                        bc(ro_s[:, t, c:c + 1], S), Alu.add)
            scr = dpool.tile([3, P, S], dt.float32r, tag="xyzscr", name="xyzscr")
            nc.sync.dma_start(scr[:].rearrange("c p s -> p c s"), xyzc[:])
            rhs6 = rpool.tile([6, P * S // 2], dt.float32r, tag="rhs6c",
                              name="rhs6c")
            scrf = scr[:].rearrange("c p s -> c (p s)")
            half = P * S // 2
            nc.sync.dma_start(rhs6[0:3, :], scrf[:, 0:half])
            nc.sync.dma_start(rhs6[3:6, :], scrf[:, half:2 * half])

            sg_all1 = gpool.tile([128, 1024], dt.float32, tag="sgall1",
                                 name="sgall1")

            pS = None
            for hf in range(4):          # 4 half-groups of 1024 cols
                rh1 = bpool.tile([128, 1024], dt.bfloat16, tag="rh1", name="rh1")
                if hf % 2 == 0:
                    pS = ppS.tile([128, 512], dt.float32, tag="pS", name="pS")
                    if SIM_SAFE:
                        nc.vector.memset(pS[:], 0.0)
                for c2 in range(2):
                    pA = ppA.tile([128, 512], dt.float32, tag="pA", name="pA")
                    mmr(pA[:], lhsT6_s[:],
                        rhs6[:, 1024 * hf + 512 * c2:1024 * hf + 512 * (c2 + 1)],
                        start=True, stop=True)
                    rsl = rh1[:, 512 * c2:512 * (c2 + 1)]
                    if (2 * hf + c2) % 2 == 0:
                        nc.vector.tensor_scalar(rsl, pA[:], b1_s[:], 0.0,
                                                Alu.add, Alu.max)
                    else:
                        nc.scalar.activation(rsl, pA[:], Act.Relu, bias=b1_s[:])
                    cc_g = 2 * (hf % 2) + c2
                    pos = 32 * cc_g
                    nc.tensor.matmul(pS[pos:pos + 2, :], w0p_s[:],
                                     rh1[:, 512 * c2:512 * (c2 + 1)],
                                     start=True, stop=True,
                                     tile_position=(0, pos))
                if hf % 2 == 1:
                    ps_i = hf // 2
                    dsl = sg_all1[:, 512 * ps_i:512 * (ps_i + 1)]
                    if ps_i == 0:
                        nc.vector.tensor_copy(dsl, pS[:])
                    else:
                        nc.scalar.copy(dsl, pS[:])
            # rows {32a+q} packed -> DRAM (q, p=32ps+8a+pl, s) -> rays
            sscr = dpool.tile([2, 64, S], dt.float32, tag="sigscr",
                              name="sigscr")
            sgv = sg_all1[:].rearrange("(a w) c -> a w c", a=4)
            for q in range(2):
                dv = sscr[q].rearrange("(ps a pl) s -> ps a (pl s)",
                                       ps=2, a=4)
                for ps in range(2):
                    nc.sync.dma_start(dv[ps],
                                      sgv[:, q, 512 * ps:512 * (ps + 1)])
            nc.sync.dma_start(h20T[:, t, :],
                              sscr[:].rearrange("q p s -> (q p) s"))

        # ================= PHASE 2: fine MLP + composite =================
        fh = P * M // 2

        def phase2_tile(t):
            xyzm = wpool.tile([P, 3, M], dt.bfloat16, tag="xyzm", name="xyzm")
            xyzmf = wpool.tile([P, 3, M], dt.float32, tag="xyzmf",
                               name="xyzmf")
            for c in range(3):
                nc.gpsimd.tensor_tensor(xyzmf[:, c, :], Z3[:, t, :],
                                        bc(rd_s[:, t, c:c + 1], M), Alu.mult)
                nc.gpsimd.tensor_tensor(xyzm[:, c, :], xyzmf[:, c, :],
                                        bc(ro_s[:, t, c:c + 1], M), Alu.add)
            scr2 = dpool.tile([3, P, M], dt.bfloat16, tag="xyzscr2",
                              name="xyzscr2")
            nc.sync.dma_start(scr2[:].rearrange("c p s -> p c s"), xyzm[:])
            rdt6_sb = rpoolf.tile([6, fh], dt.bfloat16, tag="rdt6",
                                  name="rdt6")
            nc.sync.dma_start(rdt6_sb[:], rdT6_in[t])
            rhs6f = rpoolf.tile([6, fh], dt.bfloat16, tag="rhs6f",
                                name="rhs6f")
            scr2f = scr2[:].rearrange("c p s -> c (p s)")
            nc.sync.dma_start(rhs6f[0:3, :], scr2f[:, 0:fh])
            nc.sync.dma_start(rhs6f[3:6, :], scr2f[:, fh:2 * fh])

            sg_all2 = gpool.tile([128, 2048], dt.bfloat16, tag="sgall2",
                                 name="sgall2")
            rgb_all = gpool.tile([6, 8192], dt.bfloat16, tag="rgball",
                                 name="rgball")

            for g in range(4):
                pS = ppS.tile([128, 512], dt.float32, tag="pS", name="pS")
                if SIM_SAFE:
                    nc.vector.memset(pS[:], 0.0)
                for hf in range(2):
                    base = g * 2048 + hf * 1024
                    rh1f = bpool.tile([128, 1024], dt.bfloat16, tag="rh1",
                                      name="rh1")
                    ch1 = bpool.tile([128, 1024], dt.bfloat16, tag="ch1",
                                     name="ch1")
                    for c2 in range(2):
                        cbase = base + 512 * c2
                        pA = ppA.tile([128, 512], dt.float32, tag="pA",
                                      name="pA")
                        nc.tensor.matmul(pA[:], lhsT6b_s[:],
                                         rhs6f[:, cbase:cbase + 512],
                                         start=True, stop=True)
                        rsl = rh1f[:, 512 * c2:512 * (c2 + 1)]
                        if (2 * hf + c2) % 2 == 0:
                            nc.vector.tensor_scalar(rsl, pA[:], b1_s[:], 0.0,
                                                    Alu.add, Alu.max)
                        else:
                            nc.scalar.activation(rsl, pA[:], Act.Relu,
                                                 bias=b1_s[:])
                        cc4 = 2 * hf + c2
                        pos = 32 * cc4
                        nc.tensor.matmul(pS[pos:pos + 2, :], w0p_s[:],
                                         rh1f[:, 512 * c2:512 * (c2 + 1)],
                                         start=True, stop=True,
                                         tile_position=(0, pos))
                        # dterm (K=6 on s-broadcast dirs) + geo into one
                        # psum chunk
                        pG = ppG.tile([128, 512], dt.float32, tag="pG",
                                      name="pG")
                        nc.tensor.matmul(pG[:], dlhs6_s[:],
                                         rdt6_sb[:, cbase:cbase + 512],
                                         start=True, stop=False)
                        nc.tensor.matmul(pG[:], wgc_s[:],
                                         rh1f[:, 512 * c2:512 * (c2 + 1)],
                                         start=False, stop=True)
                        csl = ch1[:, 512 * c2:512 * (c2 + 1)]
                        if (2 * hf + c2) % 2 == 0:
                            nc.scalar.activation(csl, pG[:], Act.Relu,
                                                 bias=bch1_s[:])
                        else:
                            nc.vector.tensor_scalar(csl, pG[:], bch1_s[:],
                                                    0.0, Alu.add, Alu.max)
                        pC = ppC.tile([6, 512], dt.float32, tag="pC",
                                      name="pC")
                        nc.tensor.matmul(
                            pC[:], wc2_s[:], ch1[:, 512 * c2:512 * (c2 + 1)],
                            start=True, stop=True)
                        osl = rgb_all[:, cbase:cbase + 512]
                        if (2 * hf + c2) % 2 == 0:
                            nc.scalar.activation(osl, pC[:], Act.Identity,
                                                 bias=bc2_s[:])
                        else:
                            nc.vector.tensor_scalar(osl, pC[:], bc2_s[:],
                                                    None, Alu.add)
                # sigma eviction (packed rows {32a+q})
                dsl2 = sg_all2[:, 512 * g:512 * (g + 1)]
                if g % 2 == 0:
                    nc.vector.tensor_copy(dsl2, pS[:])
                else:
                    nc.scalar.copy(dsl2, pS[:])

            # ---- scramble sigma + rgb to rays layout via DRAM bounce ----
            sscr2 = dpool.tile([2, 64, M], dt.bfloat16, tag="sigscr2",
                               name="sigscr2")
            sgv2 = sg_all2[:].rearrange("(a w) c -> a w c", a=4)
            for q in range(2):
                dv2 = sscr2[q].rearrange("(g a pl) s -> a g (pl s)",
                                         g=4, a=4)
                nc.sync.dma_start(dv2, sgv2[:, q, :])
            rscr = dpool.tile([2, 64, 3, M], dt.bfloat16, tag="rgbscr",
                              name="rgbscr")
            for q in range(2):
                nc.sync.dma_start(rscr[q].rearrange("p c s -> c p s"),
                                  rgb_all[3 * q:3 * (q + 1), :])

            return sscr2, rscr

        # ---- composite in rays layout, batched over the 2-tile round ----
        def wt(shape, tag, dtype=dt.float32):
            return kpool.tile(shape, dtype, tag=tag, name=tag)

        def phase2_comp(k, scrs):
            t0 = 2 * k
            h20m = wt([P, 2, M], "h20m", dt.bfloat16)
            rgbp = wt([P, 2, 3, M], "rgbp", dt.bfloat16)
            for i in range(2):
                ss, rs = scrs[i]
                nc.sync.dma_start(h20m[:, i, :],
                                  ss[:].rearrange("q p s -> (q p) s"))
                nc.sync.dma_start(rgbp[:, i, :, :],
                                  rs[:].rearrange("q p c s -> (q p) (c s)"))
            sigm = wt([P, 2, M], "sigm")
            nc.scalar.activation(sigm[:].rearrange("p b s -> p (b s)"),
                                 h20m[:].rearrange("p b s -> p (b s)"),
                                 Act.Exp, bias=bd2_0c)
            dsg2 = wt([P, 2, M], "dsg2")
            nc.vector.tensor_tensor(dsg2[:], dl3[:, t0:t0 + 2, :], sigm[:],
                                    Alu.mult)
            em2 = wt([P, 2, M], "em2")
            nc.scalar.activation(em2[:].rearrange("p b s -> p (b s)"),
                                 dsg2[:].rearrange("p b s -> p (b s)"),
                                 Act.Exp, scale=-1.0)
            sb2 = wt([P, 2, M], "sb2")
            nc.vector.memset(sb2[:, :, 0:1], 0.0)
            nc.scalar.activation(sb2[:, :, 1:M], em2[:, :, 0:M - 1],
                                 Act.Identity, bias=e15_c)
            Tm = wt([P, 2, M], "Tm")
            nc.vector.tensor_tensor_scan(Tm[:].rearrange("p b s -> p (b s)"),
                                         sb2[:].rearrange("p b s -> p (b s)"),
                                         one2M_s[:], 0.0, Alu.mult, Alu.add)
            alpm = wt([P, 2, M], "alpm")
            nc.scalar.activation(alpm[:].rearrange("p b s -> p (b s)"),
                                 em2[:].rearrange("p b s -> p (b s)"),
                                 Act.Identity, scale=-1.0, bias=ones_c)
            wm = wt([P, 2, M], "wm")
            nc.vector.tensor_tensor(wm[:], alpm[:], Tm[:], Alu.mult)
            wsum = wt([P, 2], "wsum")
            nc.vector.tensor_reduce(wsum[:].rearrange("p (b o) -> p b o", o=1),
                                    wm[:], AxX, Alu.add)
            wmm = wt([P, 2, M], "wmm")
            nc.vector.scalar_tensor_tensor(wmm[:], wm[:], 1e-4, wm[:],
                                           Alu.is_gt, Alu.mult)
            erg = wt([P, 2, 3, M], "erg", dt.bfloat16)
            nc.scalar.activation(erg[:].rearrange("p b c s -> p (b c s)"),
                                 rgbp[:].rearrange("p b c s -> p (b c s)"),
                                 Act.Sigmoid)
            nc.vector.tensor_tensor(
                erg[:], erg[:],
                wmm[:].rearrange("p b (o s) -> p b o s", o=1)
                .broadcast_to((P, 2, 3, M)), Alu.mult)
            img2 = wt([P, 2, 3], "img2")
            nc.vector.tensor_reduce(
                img2[:].rearrange("p b (c o) -> p b c o", o=1),
                erg[:], AxX, Alu.add)
            bgt = wt([P, 2, 3], "bgt")
            nc.vector.tensor_scalar(bgt[:],
                                    wsum[:].rearrange("p (b o) -> p b o", o=1)
                                    .broadcast_to((P, 2, 3)),
                                    -1.0, 1.0, Alu.mult, Alu.add)
            nc.vector.tensor_tensor(
                bgt[:], bgt[:],
                bg_s[:].rearrange("p (o c) -> p o c", o=1)
                .broadcast_to((P, 2, 3)), Alu.mult)
            nc.vector.tensor_tensor(img2[:], img2[:], bgt[:], Alu.add)
            nc.sync.dma_start(img_out[:, t0:t0 + 2, :], img2[:])

        # ============ software-pipelined schedule ============
        # ready phase-2 work is emitted BEFORE the next machinery chain so
        # the in-order engine queues don't head-of-line block on its waits
        for t in range(T):
            phase1_tile(t)
            if t % MB == MB - 1:
                k = t // MB
                do_mach(k)
                if k >= 1:
                    s0 = phase2_tile(2 * (k - 1))
                    s1 = phase2_tile(2 * (k - 1) + 1)
                    phase2_comp(k - 1, (s0, s1))
        s0 = phase2_tile(T - 2)
        s1 = phase2_tile(T - 1)
        phase2_comp(NG - 1, (s0, s1))

    nc.compile()
    return nc


def _host_constants(inputs):
    Wd1 = np.asarray(inputs["Wd1"], np.float32)
    bd1 = np.asarray(inputs["bd1"], np.float32)
    Wd2 = np.asarray(inputs["Wd2"], np.float32)
    bd2 = np.asarray(inputs["bd2"], np.float32)
    Wc1 = np.asarray(inputs["Wc1"], np.float32)
    bc1 = np.asarray(inputs["bc1"], np.float32)
    Wc2 = np.asarray(inputs["Wc2"], np.float32)
    bc2 = np.asarray(inputs["bc2"], np.float32)
    tval = float(np.asarray(inputs["time"]).reshape(()))

    W1 = Wd1[:3]
    b1p = bd1 + tval * Wd1[3]
    w0 = Wd2[:, 0:1]
    Wgc = (Wd2[:, 1:].astype(np.float64) @ Wc1[3:].astype(np.float64)) \
        .astype(np.float32)
    bgc = (bd2[1:].astype(np.float64) @ Wc1[3:].astype(np.float64)) \
        .astype(np.float32)
    bd2_0 = float(bd2[0])

    lhsT6 = np.zeros((6, 128), np.float32)
    lhsT6[0:3, 0:64] = W1
    lhsT6[3:6, 64:128] = W1
    b1col = np.concatenate([b1p, b1p]).reshape(128, 1).astype(np.float32)

    w0pair = np.zeros((128, 2), np.float32)
    w0pair[0:64, 0:1] = w0
    w0pair[64:128, 1:2] = w0

    wgcpair = np.zeros((128, 128), np.float32)
    wgcpair[0:64, 0:64] = Wgc
    wgcpair[64:128, 64:128] = Wgc

    wc2pair = np.zeros((128, 6), np.float32)
    wc2pair[0:64, 0:3] = Wc2
    wc2pair[64:128, 3:6] = Wc2

    dlhs6 = np.zeros((6, 128), np.float32)
    dlhs6[0:3, 0:64] = Wc1[:3]
    dlhs6[3:6, 64:128] = Wc1[:3]
    bch1col = np.concatenate([bc1 + bgc, bc1 + bgc]).reshape(128, 1) \
        .astype(np.float32)

    bc2col = np.concatenate([bc2, bc2]).reshape(6, 1).astype(np.float32)


    v = np.linspace(0.0, 1.0, S, dtype=np.float32)

    jM = np.arange(T * M)
    j62 = np.arange(T * 62)
    jS = np.arange(T * S)
    segmaskM = (jM % M != 0).astype(np.float32)
    iop1T = (jM % M + 1).astype(np.float32)
    seg256T = (256 * ((jM // M) % MB)).astype(np.float32)
    segmask62 = (j62 % 62 != 0).astype(np.float32)
    iev62T = (2 * (j62 % 62) + 256 * ((j62 // 62) % MB)).astype(np.float32)
    oneSst = (jS % S == 0).astype(np.float32)

    def rep(row):
        return np.broadcast_to(row, (P,) + row.shape).copy()

    return {
        "c_all": np.concatenate([
            rep(v),
            rep(np.array([1.0, 1e-9, 1e-15, 1e-5, -1.0 / 128.0,
                          16777216.0, -16777216.0, -8003.0], np.float32)),
            rep(segmaskM), rep(iop1T), rep(seg256T),
            rep(segmask62), rep(iev62T), rep(oneSst),
            rep((np.arange(2 * M) % M == 0).astype(np.float32)),
            np.broadcast_to(np.asarray(inputs["background_color"],
                                       np.float32), (P, 3)).copy(),
            np.broadcast_to(np.array([bd2_0, 0, 0, 0], np.float32),
                            (P, 4)).copy(),
        ], axis=1),
        "wbig": np.concatenate([w0pair, wgcpair, wc2pair], axis=1)
        .astype(ml_dtypes.bfloat16),
        "lb6": np.concatenate([lhsT6, dlhs6], axis=1)
        .astype(ml_dtypes.bfloat16),
        "bcols": np.concatenate([b1col, bch1col], axis=1),
        "lhsT6": lhsT6,
        "bc2col": bc2col,
    }


def kernel(**inputs):
    global _BUILT
    assert int(inputs["num_steps"]) == S
    assert int(inputs["upsample_steps"]) == U

    if _BUILT is None:
        _BUILT = _build()
    nc = _BUILT

    consts = _host_constants(inputs)
    ro = np.asarray(inputs["rays_o"], np.float32).reshape(NRAYS, 3)
    rd = np.asarray(inputs["rays_d"], np.float32).reshape(NRAYS, 3)

    in_maps = []
    for c in range(NCORES):
        sl_o = ro[c * R:(c + 1) * R].reshape(T, P, 3)
        sl_d = rd[c * R:(c + 1) * R].reshape(T, P, 3)
        rdT6 = np.empty((T, 6, P // 2, M), np.float32)
        rdT6[:, 0:3] = sl_d[:, :P // 2].transpose(0, 2, 1)[..., None]
        rdT6[:, 3:6] = sl_d[:, P // 2:].transpose(0, 2, 1)[..., None]
        m = dict(consts)
        m["rdT6_k"] = rdT6.reshape(T, 6, P // 2 * M).astype(ml_dtypes.bfloat16)
        m["c_all_k"] = np.concatenate([
            consts["c_all"],
            sl_o.transpose(1, 0, 2).reshape(P, T * 3),
            sl_d.transpose(1, 0, 2).reshape(P, T * 3)], axis=1)
        del m["c_all"]
        in_maps.append(m)

    res = run_bass_kernel_spmd(nc, in_maps, core_ids=list(range(NCORES)))
    global LAST_RESULT
    LAST_RESULT = res
    outs = []
    for c in range(NCORES):
        img = res.results[c]["img_k"]
        outs.append(img.transpose(1, 0, 2).reshape(R, 3))
    return np.concatenate(outs, 0).reshape(1, NRAYS, 3)

